# revision 33
# baseline (speedup 1.0000x reference)
"""GuidedAttentionLoss on Trainium2 — 8 NeuronCores, per-core-specialized
diagonal-band gather kernels.

loss = mean(attention_weights * mask), mask[b,i,j] =
    (i < out_len_b) & (j < in_len_b) ? exp(-(j - floor(i/out*in))^2 / (2*0.4^2)) : 0

With sigma=0.4 the Gaussian underflows to exactly 0 in f32 beyond
|j - ideal_i| ~ 4.6, so per valid row only a ~9-wide band of columns
contributes. Each core gets 8 whole batches (greedy-balanced by cost) and its
OWN compiled program specialized to them: per batch a quantized shear line
sigma(i) = a2*(i%128) + at*(i//128) + b tracks ideal(i), and a single 3-dim
DMA access pattern [[400+a2,128],[51200+at,nt],[1,W]] gathers the whole
batch's band ([128 rows/tile] x [W cols], nt tiles) in ONE DMA instruction,
with W fitted exactly (rows past out_len get center=+1e4 and die in the
exp underflow). Per segment:
    path A: d2[:, t] = ACT Square(-w + center_t)   (per-tile bias, no sub)
    path B: d = w - center (DVE, broadcast APs); d2 = ACT Square(d)
    g = ACT Exp(-3.125*d2);  acc[:,s] += g*attn  (DVE stt accum)
Paths are chosen per segment to balance ACT vs DVE. Out-of-range garbage
(front spill j<0, j >= min(in,400)) is not masked on device; the host
subtracts those few boundary terms exactly in f64.

The 8 programs run concurrently: each is jit-compiled for its own device and
dispatched asynchronously; results are gathered and summed on host.
"""

import numpy as np

import concourse.bacc as bacc
import concourse.bass as bass  # noqa: F401
import concourse.mybir as mybir
from concourse.ap import AP
from concourse import tile

N_CORES = 8
B, T, E = 64, 2000, 400
B_LOC = B // N_CORES
P = 128
D = 4
PADF = 512
PADB = 81920
FLAT = PADF + B_LOC * T * E + PADB
NEG_SCALE = -3.125
F32 = mybir.dt.float32
AF = mybir.ActivationFunctionType
OP = mybir.AluOpType

SEG_FIXED_NS = 900.0

_PLAN_CACHE = {}
_EXEC_CACHE = {}


def _ideal_f32(i, in_len, out_len):
    safe_out = np.float32(max(float(out_len), 1.0))
    return np.floor((i.astype(np.float32) / safe_out) * np.float32(in_len)).astype(
        np.float32
    )


def _dma_row_ns(W):
    by = 4 * W
    mult = 2.0 if by < 512 else 1.0
    return max(by * mult / 22.5, 7.0)


class _Seg:
    __slots__ = ("g", "t0", "nt", "W", "a2", "at", "b", "sigma", "path_a")

    def key(self):
        return (self.g, self.t0, self.nt, self.W, self.a2, self.at, self.b,
                self.path_a)


class _BatchCtx:
    """Row band data for a single batch."""

    def __init__(self, b, il, ol):
        self.b = b
        o, n = int(ol[b]), int(il[b])
        self.out = min(o, T)
        self.ntt = (self.out + P - 1) // P
        rows = self.ntt * P
        i = np.arange(rows)
        self.valid = i < self.out
        idl = _ideal_f32(i, n, o).astype(np.float64)
        self.A = np.maximum(0.0, idl - D)
        self.Bb = np.minimum(n - 1, idl + D)
        self.slope = il[b] / max(ol[b], 1)

    def fit(self, t0, t1):
        rows = (t1 - t0) * P
        sl = slice(t0 * P, t1 * P)
        anyv = self.valid[sl]
        if not anyv.any():
            return None
        Amin = self.A[sl]
        Bmax = self.Bb[sl]
        rr = np.arange(rows)
        t_idx = rr // P
        p = rr % P
        s = self.slope
        cands = set()
        for f1 in (np.floor, np.round):
            a2 = int(f1(s))
            for f3 in (np.floor, np.round):
                at3 = int(f3(128 * s))
                for dat in (-1, 0, 1):
                    cands.add((a2, at3 + dat))
        cands.add((0, 0))
        best = None
        for a2, at in cands:
            sig0 = a2 * p + at * t_idx
            b_off = int(np.floor((Amin - sig0)[anyv].min()))
            W = int(np.ceil((Bmax - sig0)[anyv].max() - b_off)) + 1
            if best is None or W < best[0]:
                best = (W, a2, at, b_off)
        return best

    def seg_cost(self, t0, t1):
        f = self.fit(t0, t1)
        if f is None:
            return 0.0, None
        W, a2, at, b_off = f
        nt = t1 - t0
        best = None
        for Wc in {W, 128} if 64 < W < 128 else {W}:
            fw = nt * Wc
            dma = nt * P * _dma_row_ns(Wc) / 16.0
            act = 1.67 * fw + 192 * nt
            dve = 1.042 * fw
            cost = max(dma, act, dve) + 0.25 * (act + dve) + SEG_FIXED_NS
            if best is None or cost < best[0]:
                best = (cost, (Wc, a2, at, b_off))
        return best

    def plan(self):
        nt = self.ntt
        icost = {}
        ifit = {}
        for t0 in range(nt):
            for t1 in range(t0 + 1, nt + 1):
                c, f = self.seg_cost(t0, t1)
                icost[(t0, t1)] = c
                ifit[(t0, t1)] = f
        INF = float("inf")
        dp = [INF] * (nt + 1)
        prev = [0] * (nt + 1)
        dp[0] = 0.0
        for t1 in range(1, nt + 1):
            for t0 in range(t1):
                c = dp[t0] + icost[(t0, t1)]
                if c < dp[t1]:
                    dp[t1] = c
                    prev[t1] = t0
        cuts = []
        t = nt
        while t > 0:
            t0 = prev[t]
            cuts.append((t0, t))
            t = t0
        cuts.reverse()
        out = [(t0, t1, ifit[(t0, t1)]) for t0, t1 in cuts if ifit[(t0, t1)]]
        return dp[nt], out


def _build_schedule(input_lengths, output_lengths):
    """Returns assign[c][g] = batch at core c slot g, and per-core seg lists."""
    il = np.asarray(input_lengths, dtype=np.int64)
    ol = np.asarray(output_lengths, dtype=np.int64)
    ctxs = [_BatchCtx(b, il, ol) for b in range(B)]
    plans = [c.plan() for c in ctxs]
    order = sorted(range(B), key=lambda b: -plans[b][0])
    loads = [0.0] * N_CORES
    slots = [[] for _ in range(N_CORES)]
    for b in order:
        c = min(
            (c for c in range(N_CORES) if len(slots[c]) < B_LOC),
            key=lambda c: loads[c],
        )
        slots[c].append(b)
        loads[c] += plans[b][0]
    assign = [slots[c] for c in range(N_CORES)]

    core_segs = []
    for c in range(N_CORES):
        segs = []
        for g, b in enumerate(assign[c]):
            for t0, t1, (W, a2, at, b_off) in plans[b][1]:
                seg = _Seg()
                seg.g = g
                seg.t0 = t0
                seg.nt = t1 - t0
                seg.W = W
                seg.a2 = a2
                seg.at = at
                seg.b = b_off
                rr = np.arange(seg.nt * P)
                seg.sigma = a2 * (rr % P) + at * (rr // P) + b_off
                segs.append(seg)
        # path A/B to balance ACT vs DVE within this core
        act_ns = dve_ns = 0.0
        for seg in sorted(segs, key=lambda s: -s.nt * s.W):
            fw = seg.nt * seg.W
            a_act = 1.67 * fw + 192 * (seg.nt + 1)
            a_dve = 1.042 * fw + 80
            b_act = 1.67 * fw + 384
            b_dve = 2.084 * fw + 245
            if max(act_ns + a_act, dve_ns + a_dve) <= max(
                act_ns + b_act, dve_ns + b_dve
            ):
                seg.path_a = True
                act_ns += a_act
                dve_ns += a_dve
            else:
                seg.path_a = False
                act_ns += b_act
                dve_ns += b_dve
        _coverage_check(segs, assign[c], il, ol)
        core_segs.append(segs)
    return assign, core_segs


def _coverage_check(segs, assign_c, il, ol):
    covered = {}
    for seg in segs:
        b = assign_c[seg.g]
        o, n = int(ol[b]), int(il[b])
        rows = seg.nt * P
        i = seg.t0 * P + np.arange(rows)
        v = i < min(o, T)
        if v.any():
            idl = _ideal_f32(i, n, o).astype(np.float64)
            A = np.maximum(0.0, idl - D)
            Bb = np.minimum(n - 1, idl + D)
            ok = (~v) | ((seg.sigma <= A) & (Bb < seg.sigma + seg.W))
            assert ok.all(), (seg.g, b, np.where(~ok)[0][:5])
        base = seg.g * T * E + i * E + seg.sigma
        assert (PADF + base).min() >= 0
        assert (PADF + base + seg.W).max() <= FLAT
        cv = covered.setdefault(seg.g, np.zeros(T, bool))
        cv[np.clip(i[v], 0, T - 1)] = True
    for g, b in enumerate(assign_c):
        o = min(int(ol[b]), T)
        cv = covered.get(g, np.zeros(T, bool))
        assert cv[:o].all(), (g, b, int(np.argmin(cv[:o])))


def _build_nc(segs):
    ntt = sum(s.nt for s in segs)
    nseg = len(segs)
    nc = bacc.Bacc(None, target_bir_lowering=False)
    attn = nc.declare_dram_parameter("attn", [FLAT], F32, isOutput=False)
    center_d = nc.declare_dram_parameter("center", [P, ntt], F32, isOutput=False)
    acc_d = nc.declare_dram_parameter("acc", [P, nseg], F32, isOutput=True)

    with tile.TileContext(nc) as tc:
        with (
            tc.tile_pool(name="const", bufs=1) as const_pool,
            tc.tile_pool(name="at", bufs=2) as at_pool,
            tc.tile_pool(name="d", bufs=2) as d_pool,
            tc.tile_pool(name="g", bufs=2) as g_pool,
            tc.tile_pool(name="junk", bufs=2) as junk_pool,
        ):
            w_i32 = const_pool.tile([P, E], mybir.dt.int32, tag="w_i32")
            w_f32 = const_pool.tile([P, E], F32, tag="w_f32")
            center = const_pool.tile([P, ntt], F32, tag="center")
            acc = const_pool.tile([P, nseg], F32, tag="acc")

            nc.gpsimd.iota(w_i32[:], pattern=[[1, E]], base=0, channel_multiplier=0)
            nc.vector.tensor_copy(w_f32[:], w_i32[:])
            nc.gpsimd.memset(acc[:], 0.0)

            # center table first: every compute op depends on it
            nc.sync.dma_start(out=center[:], in_=center_d[:])

            # group segments into chunks; one Square/Exp/reduce per chunk
            chunks = []
            cur = []
            cfw = 0
            for si, seg in enumerate(segs):
                cur.append(si)
                cfw += seg.nt * seg.W
                if cfw >= 640:
                    chunks.append(cur)
                    cur = []
                    cfw = 0
            if cur:
                chunks.append(cur)

            k0s = np.cumsum([0] + [s.nt for s in segs])
            for ci, chunk in enumerate(chunks):
                fwc = sum(segs[si].nt * segs[si].W for si in chunk)
                at = at_pool.tile([P, fwc], F32, tag="at")
                d = d_pool.tile([P, fwc], F32, tag="d")
                off = 0
                for si in chunk:
                    seg = segs[si]
                    nt, W = seg.nt, seg.W
                    src = AP(
                        attn[:].tensor,
                        PADF + seg.g * T * E + seg.t0 * P * E + seg.b,
                        [[E + seg.a2, P], [P * E + seg.at, nt], [1, W]],
                    )
                    dst = at[:, off : off + nt * W]
                    eng = nc.sync if si % 2 == 0 else nc.gpsimd
                    eng.dma_start(
                        out=AP(dst.tensor, dst.offset,
                               [dst.ap[0], [W, nt], [1, W]]),
                        in_=src,
                    )
                    # d = w - center (broadcast APs)
                    k0 = int(k0s[si])
                    wap = w_f32[:, 0:W]
                    w_b = AP(wap.tensor, wap.offset,
                             [wap.ap[0], [0, nt], [1, W]])
                    cap = center[:, k0 : k0 + nt]
                    c_b = AP(cap.tensor, cap.offset,
                             [cap.ap[0], [1, nt], [0, W]])
                    dsl = d[:, off : off + nt * W]
                    d3 = AP(dsl.tensor, dsl.offset,
                            [dsl.ap[0], [W, nt], [1, W]])
                    nc.vector.tensor_tensor(d3, w_b, c_b, OP.subtract)
                    off += nt * W
                d2 = junk_pool.tile([P, fwc], F32, tag="d2")
                nc.scalar.activation(d2[:], d[:], AF.Square)
                gt = g_pool.tile([P, fwc], F32, tag="gt")
                nc.scalar.activation(gt[:], d2[:], AF.Exp, scale=NEG_SCALE)
                jk = junk_pool.tile([P, fwc], F32, tag="jk")
                nc.vector.scalar_tensor_tensor(
                    jk[:], gt[:], 1.0, at[:], OP.mult, OP.mult,
                    accum_out=acc[:, ci : ci + 1],
                )
            nc.gpsimd.dma_start(out=acc_d[:], in_=acc[:])
    return nc


def _make_tables(il, ol, assign_c, segs):
    ntt = sum(s.nt for s in segs)
    center = np.full((P, ntt), 1e4, np.float32)
    k0 = 0
    for seg in segs:
        b = assign_c[seg.g]
        o, n = int(ol[b]), int(il[b])
        rows = seg.nt * P
        i = seg.t0 * P + np.arange(rows)
        idl = _ideal_f32(i, n, o)
        validr = i < min(o, T)
        cen = np.where(validr, idl - seg.sigma.astype(np.float32), np.float32(1e4))
        center[:, k0 : k0 + seg.nt] = cen.reshape(seg.nt, P).T
        k0 += seg.nt
    return {"center": center}


def _garbage_correction(in_maps, il, ol, assign, core_segs):
    M = 24
    corr = 0.0
    for c in range(N_CORES):
        flat = in_maps[c]["attn"]
        for seg in core_segs[c]:
            b = assign[c][seg.g]
            o, n = int(ol[b]), int(il[b])
            lim = min(n, E)
            rows = seg.nt * P
            i = seg.t0 * P + np.arange(rows)
            validr = i < min(o, T)
            idl = _ideal_f32(i, n, o).astype(np.float64)
            sg = seg.sigma
            fr = validr & (
                ((sg < 0) & (idl <= M)) | ((sg + seg.W > lim) & (idl >= lim - M))
            )
            if not fr.any():
                continue
            ii = i[fr]
            j = sg[fr][:, None] + np.arange(seg.W)[None, :]
            dd = j - idl[fr][:, None]
            bad = ((j < 0) | (j >= lim)) & (np.abs(dd) <= M)
            if not bad.any():
                continue
            addr = PADF + seg.g * T * E + ii[:, None] * E + j
            vals = flat[addr[bad]].astype(np.float64)
            corr += float(np.sum(np.exp(-3.125 * dd[bad] ** 2) * vals))
    return corr


def _get_compiled(c, segs, in_map):
    """jit-compile core c's program for device c; cache across calls."""
    import jax
    from concourse import bass2jax
    from concourse.bass2jax import _bass_exec_p

    key = (c, tuple(s.key() for s in segs))
    if key in _EXEC_CACHE:
        return _EXEC_CACHE[key]

    bass2jax.install_neuronx_cc_hook()
    nc = _build_nc(segs)
    if not nc.is_finalized():
        nc.finalize()

    in_names, out_names, out_avals, zero_outs = [], [], [], []
    for alloc in nc.m.functions[0].allocations:
        if not isinstance(alloc, mybir.MemoryLocationSet):
            continue
        name = alloc.memorylocations[0].name
        if alloc.kind == "ExternalInput":
            in_names.append(name)
        elif alloc.kind == "ExternalOutput":
            out_names.append(name)
            shape = tuple(alloc.tensor_shape)
            dtype = mybir.dt.np(alloc.dtype)
            out_avals.append(jax.core.ShapedArray(shape, dtype))
            zero_outs.append(np.zeros(shape, dtype))
    n_params = len(in_names)
    all_names = in_names + out_names
    donate = tuple(range(n_params, n_params + len(out_names)))

    def _body(*args):
        outs = _bass_exec_p.bind(
            *args,
            out_avals=tuple(out_avals),
            in_names=tuple(all_names),
            out_names=tuple(out_names),
            lowering_input_output_aliases=(),
            sim_require_finite=True,
            sim_require_nnan=True,
            nc=nc,
        )
        return tuple(outs)

    dev = jax.devices()[c]
    with jax.default_device(dev):
        jf = jax.jit(_body, donate_argnums=donate, keep_unused=True)
        args = _core_args(nc, in_names, zero_outs, in_map, c)
        comp = jf.lower(*args).compile()
    entry = (comp, nc, in_names, out_names, zero_outs)
    _EXEC_CACHE[key] = entry
    return entry


def _core_args(nc, in_names, zero_outs, in_map, c):
    im = dict(in_map)
    if nc.partition_id_tensor is not None:
        im[nc.partition_id_tensor.name] = np.array([[c]], dtype=np.uint32)
    return [np.asarray(im[n]) for n in in_names] + [z.copy() for z in zero_outs]


def _run(attention_weights, input_lengths, output_lengths, ntff_hook=None):
    attention_weights = np.ascontiguousarray(attention_weights, dtype=np.float32)
    il = np.asarray(input_lengths, dtype=np.int64)
    ol = np.asarray(output_lengths, dtype=np.int64)
    assign, core_segs = _build_schedule(il, ol)
    in_maps = []
    for c in range(N_CORES):
        flat = np.empty(FLAT, np.float32)
        flat[:PADF] = 0.0
        flat[PADF : PADF + B_LOC * T * E] = attention_weights[assign[c]].reshape(-1)
        flat[PADF + B_LOC * T * E :] = 0.0
        in_maps.append(
            {"attn": flat, **_make_tables(il, ol, assign[c], core_segs[c])}
        )

    entries = [
        _get_compiled(c, core_segs[c], in_maps[c]) for c in range(N_CORES)
    ]

    def _dispatch():
        futs = []
        for c, (comp, nc, in_names, out_names, zero_outs) in enumerate(entries):
            args = _core_args(nc, in_names, zero_outs, in_maps[c], c)
            futs.append((comp(*args), out_names))
        return [
            {name: np.asarray(v) for name, v in zip(out_names, outs)}
            for outs, out_names in futs
        ]

    if ntff_hook is not None:
        with ntff_hook:
            results = _dispatch()
    else:
        results = _dispatch()

    total = sum(float(r["acc"].sum(dtype=np.float64)) for r in results)
    total -= _garbage_correction(in_maps, il, ol, assign, core_segs)
    return np.float32(total / float(B * T * E)), results


def kernel(attention_weights, input_lengths, output_lengths):
    out, _ = _run(attention_weights, input_lengths, output_lengths)
    return out


# revision 34
# speedup vs baseline: 1.0680x; 1.0680x over previous
"""GuidedAttentionLoss on Trainium2 — 8 NeuronCores, per-core-specialized
diagonal-band gather kernels.

loss = mean(attention_weights * mask), mask[b,i,j] =
    (i < out_len_b) & (j < in_len_b) ? exp(-(j - floor(i/out*in))^2 / (2*0.4^2)) : 0

With sigma=0.4 the Gaussian underflows to exactly 0 in f32 beyond
|j - ideal_i| ~ 4.6, so per valid row only a ~9-wide band of columns
contributes. Each core gets 8 whole batches (greedy-balanced by cost) and its
OWN compiled program specialized to them: per batch a quantized shear line
sigma(i) = a2*(i%128) + at*(i//128) + b tracks ideal(i), and a single 3-dim
DMA access pattern [[400+a2,128],[51200+at,nt],[1,W]] gathers the whole
batch's band ([128 rows/tile] x [W cols], nt tiles) in ONE DMA instruction,
with W fitted exactly (rows past out_len get center=+1e4 and die in the
exp underflow). Per segment:
    path A: d2[:, t] = ACT Square(-w + center_t)   (per-tile bias, no sub)
    path B: d = w - center (DVE, broadcast APs); d2 = ACT Square(d)
    g = ACT Exp(-3.125*d2);  acc[:,s] += g*attn  (DVE stt accum)
Paths are chosen per segment to balance ACT vs DVE. Out-of-range garbage
(front spill j<0, j >= min(in,400)) is not masked on device; the host
subtracts those few boundary terms exactly in f64.

The 8 programs run concurrently: each is jit-compiled for its own device and
dispatched asynchronously; results are gathered and summed on host.
"""

import numpy as np

import concourse.bacc as bacc
import concourse.bass as bass  # noqa: F401
import concourse.mybir as mybir
from concourse.ap import AP
from concourse import tile

N_CORES = 8
B, T, E = 64, 2000, 400
B_LOC = B // N_CORES
P = 128
D = 4
PADF = 512
PADB = 81920
FLAT = PADF + B_LOC * T * E + PADB
NEG_SCALE = -3.125
F32 = mybir.dt.float32
AF = mybir.ActivationFunctionType
OP = mybir.AluOpType

SEG_FIXED_NS = 900.0

_PLAN_CACHE = {}
_EXEC_CACHE = {}


def _ideal_f32(i, in_len, out_len):
    safe_out = np.float32(max(float(out_len), 1.0))
    return np.floor((i.astype(np.float32) / safe_out) * np.float32(in_len)).astype(
        np.float32
    )


def _dma_row_ns(W):
    by = 4 * W
    mult = 2.0 if by < 512 else 1.0
    return max(by * mult / 22.5, 7.0)


class _Seg:
    __slots__ = ("g", "t0", "nt", "W", "a2", "at", "b", "sigma", "path_a")

    def key(self):
        return (self.g, self.t0, self.nt, self.W, self.a2, self.at, self.b,
                self.path_a)


class _BatchCtx:
    """Row band data for a single batch."""

    def __init__(self, b, il, ol):
        self.b = b
        o, n = int(ol[b]), int(il[b])
        self.out = min(o, T)
        self.ntt = (self.out + P - 1) // P
        rows = self.ntt * P
        i = np.arange(rows)
        self.valid = i < self.out
        idl = _ideal_f32(i, n, o).astype(np.float64)
        self.A = np.maximum(0.0, idl - D)
        self.Bb = np.minimum(n - 1, idl + D)
        self.slope = il[b] / max(ol[b], 1)

    def fit(self, t0, t1):
        rows = (t1 - t0) * P
        sl = slice(t0 * P, t1 * P)
        anyv = self.valid[sl]
        if not anyv.any():
            return None
        Amin = self.A[sl]
        Bmax = self.Bb[sl]
        rr = np.arange(rows)
        t_idx = rr // P
        p = rr % P
        s = self.slope
        cands = set()
        for f1 in (np.floor, np.round):
            a2 = int(f1(s))
            for f3 in (np.floor, np.round):
                at3 = int(f3(128 * s))
                for dat in (-1, 0, 1):
                    cands.add((a2, at3 + dat))
        cands.add((0, 0))
        best = None
        for a2, at in cands:
            sig0 = a2 * p + at * t_idx
            b_off = int(np.floor((Amin - sig0)[anyv].min()))
            W = int(np.ceil((Bmax - sig0)[anyv].max() - b_off)) + 1
            if best is None or W < best[0]:
                best = (W, a2, at, b_off)
        return best

    def seg_cost(self, t0, t1):
        f = self.fit(t0, t1)
        if f is None:
            return 0.0, None
        W, a2, at, b_off = f
        nt = t1 - t0
        best = None
        for Wc in {W, 128} if 64 < W < 128 else {W}:
            fw = nt * Wc
            dma = nt * P * _dma_row_ns(Wc) / 16.0
            act = 1.67 * fw + 192 * nt
            dve = 1.042 * fw
            cost = max(dma, act, dve) + 0.25 * (act + dve) + SEG_FIXED_NS
            if best is None or cost < best[0]:
                best = (cost, (Wc, a2, at, b_off))
        return best

    def plan(self):
        nt = self.ntt
        icost = {}
        ifit = {}
        for t0 in range(nt):
            for t1 in range(t0 + 1, nt + 1):
                c, f = self.seg_cost(t0, t1)
                icost[(t0, t1)] = c
                ifit[(t0, t1)] = f
        INF = float("inf")
        dp = [INF] * (nt + 1)
        prev = [0] * (nt + 1)
        dp[0] = 0.0
        for t1 in range(1, nt + 1):
            for t0 in range(t1):
                c = dp[t0] + icost[(t0, t1)]
                if c < dp[t1]:
                    dp[t1] = c
                    prev[t1] = t0
        cuts = []
        t = nt
        while t > 0:
            t0 = prev[t]
            cuts.append((t0, t))
            t = t0
        cuts.reverse()
        out = [(t0, t1, ifit[(t0, t1)]) for t0, t1 in cuts if ifit[(t0, t1)]]
        return dp[nt], out


def _build_schedule(input_lengths, output_lengths):
    """Returns assign[c][g] = batch at core c slot g, and per-core seg lists."""
    il = np.asarray(input_lengths, dtype=np.int64)
    ol = np.asarray(output_lengths, dtype=np.int64)
    ctxs = [_BatchCtx(b, il, ol) for b in range(B)]
    plans = [c.plan() for c in ctxs]
    order = sorted(range(B), key=lambda b: -plans[b][0])
    loads = [0.0] * N_CORES
    slots = [[] for _ in range(N_CORES)]
    for b in order:
        c = min(
            (c for c in range(N_CORES) if len(slots[c]) < B_LOC),
            key=lambda c: loads[c],
        )
        slots[c].append(b)
        loads[c] += plans[b][0]
    assign = [slots[c] for c in range(N_CORES)]

    core_segs = []
    for c in range(N_CORES):
        segs = []
        for g, b in enumerate(assign[c]):
            for t0, t1, (W, a2, at, b_off) in plans[b][1]:
                seg = _Seg()
                seg.g = g
                seg.t0 = t0
                seg.nt = t1 - t0
                seg.W = W
                seg.a2 = a2
                seg.at = at
                seg.b = b_off
                rr = np.arange(seg.nt * P)
                seg.sigma = a2 * (rr % P) + at * (rr // P) + b_off
                segs.append(seg)
        # path A/B to balance ACT vs DVE within this core
        act_ns = dve_ns = 0.0
        for seg in sorted(segs, key=lambda s: -s.nt * s.W):
            fw = seg.nt * seg.W
            a_act = 1.67 * fw + 192 * (seg.nt + 1)
            a_dve = 1.042 * fw + 80
            b_act = 1.67 * fw + 384
            b_dve = 2.084 * fw + 245
            if max(act_ns + a_act, dve_ns + a_dve) <= max(
                act_ns + b_act, dve_ns + b_dve
            ):
                seg.path_a = True
                act_ns += a_act
                dve_ns += a_dve
            else:
                seg.path_a = False
                act_ns += b_act
                dve_ns += b_dve
        _coverage_check(segs, assign[c], il, ol)
        core_segs.append(segs)
    return assign, core_segs


def _coverage_check(segs, assign_c, il, ol):
    covered = {}
    for seg in segs:
        b = assign_c[seg.g]
        o, n = int(ol[b]), int(il[b])
        rows = seg.nt * P
        i = seg.t0 * P + np.arange(rows)
        v = i < min(o, T)
        if v.any():
            idl = _ideal_f32(i, n, o).astype(np.float64)
            A = np.maximum(0.0, idl - D)
            Bb = np.minimum(n - 1, idl + D)
            ok = (~v) | ((seg.sigma <= A) & (Bb < seg.sigma + seg.W))
            assert ok.all(), (seg.g, b, np.where(~ok)[0][:5])
        base = seg.g * T * E + i * E + seg.sigma
        assert (PADF + base).min() >= 0
        assert (PADF + base + seg.W).max() <= FLAT
        cv = covered.setdefault(seg.g, np.zeros(T, bool))
        cv[np.clip(i[v], 0, T - 1)] = True
    for g, b in enumerate(assign_c):
        o = min(int(ol[b]), T)
        cv = covered.get(g, np.zeros(T, bool))
        assert cv[:o].all(), (g, b, int(np.argmin(cv[:o])))


def _build_nc(segs):
    ntt = sum(s.nt for s in segs)
    nseg = len(segs)
    nc = bacc.Bacc(None, target_bir_lowering=False)
    attn = nc.declare_dram_parameter("attn", [FLAT], F32, isOutput=False)
    center_d = nc.declare_dram_parameter("center", [P, ntt], F32, isOutput=False)
    acc_d = nc.declare_dram_parameter("acc", [P, nseg], F32, isOutput=True)

    with tile.TileContext(nc) as tc:
        with (
            tc.tile_pool(name="const", bufs=1) as const_pool,
            tc.tile_pool(name="at", bufs=2) as at_pool,
            tc.tile_pool(name="d", bufs=2) as d_pool,
            tc.tile_pool(name="g", bufs=2) as g_pool,
            tc.tile_pool(name="junk", bufs=2) as junk_pool,
        ):
            w_i32 = const_pool.tile([P, E], mybir.dt.int32, tag="w_i32")
            w_f32 = const_pool.tile([P, E], F32, tag="w_f32")
            center = const_pool.tile([P, ntt], F32, tag="center")
            acc = const_pool.tile([P, nseg], F32, tag="acc")

            nc.gpsimd.iota(w_i32[:], pattern=[[1, E]], base=0, channel_multiplier=0)
            nc.vector.tensor_copy(w_f32[:], w_i32[:])
            nc.gpsimd.memset(acc[:], 0.0)

            # center table first: every compute op depends on it
            nc.sync.dma_start(out=center[:], in_=center_d[:])

            # group segments into chunks; one Square/Exp/reduce per chunk
            chunks = []
            cur = []
            cfw = 0
            for si, seg in enumerate(segs):
                cur.append(si)
                cfw += seg.nt * seg.W
                if cfw >= 640:
                    chunks.append(cur)
                    cur = []
                    cfw = 0
            if cur:
                chunks.append(cur)

            k0s = np.cumsum([0] + [s.nt for s in segs])
            for ci, chunk in enumerate(chunks):
                fwc = sum(segs[si].nt * segs[si].W for si in chunk)
                at = at_pool.tile([P, fwc], F32, tag="at")
                d = d_pool.tile([P, fwc], F32, tag="d")
                off = 0
                for si in chunk:
                    seg = segs[si]
                    nt, W = seg.nt, seg.W
                    src = AP(
                        attn[:].tensor,
                        PADF + seg.g * T * E + seg.t0 * P * E + seg.b,
                        [[E + seg.a2, P], [P * E + seg.at, nt], [1, W]],
                    )
                    dst = at[:, off : off + nt * W]
                    nc.sync.dma_start(
                        out=AP(dst.tensor, dst.offset,
                               [dst.ap[0], [W, nt], [1, W]]),
                        in_=src,
                    )
                    # d = w - center (broadcast APs)
                    k0 = int(k0s[si])
                    wap = w_f32[:, 0:W]
                    w_b = AP(wap.tensor, wap.offset,
                             [wap.ap[0], [0, nt], [1, W]])
                    cap = center[:, k0 : k0 + nt]
                    c_b = AP(cap.tensor, cap.offset,
                             [cap.ap[0], [1, nt], [0, W]])
                    dsl = d[:, off : off + nt * W]
                    d3 = AP(dsl.tensor, dsl.offset,
                            [dsl.ap[0], [W, nt], [1, W]])
                    nc.vector.tensor_tensor(d3, w_b, c_b, OP.subtract)
                    off += nt * W
                d2 = junk_pool.tile([P, fwc], F32, tag="d2")
                nc.scalar.activation(d2[:], d[:], AF.Square)
                gt = g_pool.tile([P, fwc], F32, tag="gt")
                nc.scalar.activation(gt[:], d2[:], AF.Exp, scale=NEG_SCALE)
                jk = junk_pool.tile([P, fwc], F32, tag="jk")
                nc.vector.scalar_tensor_tensor(
                    jk[:], gt[:], 1.0, at[:], OP.mult, OP.mult,
                    accum_out=acc[:, ci : ci + 1],
                )
            nc.sync.dma_start(out=acc_d[:], in_=acc[:])
    return nc


def _make_tables(il, ol, assign_c, segs):
    ntt = sum(s.nt for s in segs)
    center = np.full((P, ntt), 1e4, np.float32)
    k0 = 0
    for seg in segs:
        b = assign_c[seg.g]
        o, n = int(ol[b]), int(il[b])
        rows = seg.nt * P
        i = seg.t0 * P + np.arange(rows)
        idl = _ideal_f32(i, n, o)
        validr = i < min(o, T)
        cen = np.where(validr, idl - seg.sigma.astype(np.float32), np.float32(1e4))
        center[:, k0 : k0 + seg.nt] = cen.reshape(seg.nt, P).T
        k0 += seg.nt
    return {"center": center}


def _garbage_correction(in_maps, il, ol, assign, core_segs):
    M = 24
    corr = 0.0
    for c in range(N_CORES):
        flat = in_maps[c]["attn"]
        for seg in core_segs[c]:
            b = assign[c][seg.g]
            o, n = int(ol[b]), int(il[b])
            lim = min(n, E)
            rows = seg.nt * P
            i = seg.t0 * P + np.arange(rows)
            validr = i < min(o, T)
            idl = _ideal_f32(i, n, o).astype(np.float64)
            sg = seg.sigma
            fr = validr & (
                ((sg < 0) & (idl <= M)) | ((sg + seg.W > lim) & (idl >= lim - M))
            )
            if not fr.any():
                continue
            ii = i[fr]
            j = sg[fr][:, None] + np.arange(seg.W)[None, :]
            dd = j - idl[fr][:, None]
            bad = ((j < 0) | (j >= lim)) & (np.abs(dd) <= M)
            if not bad.any():
                continue
            addr = PADF + seg.g * T * E + ii[:, None] * E + j
            vals = flat[addr[bad]].astype(np.float64)
            corr += float(np.sum(np.exp(-3.125 * dd[bad] ** 2) * vals))
    return corr


def _get_compiled(c, segs, in_map):
    """jit-compile core c's program for device c; cache across calls."""
    import jax
    from concourse import bass2jax
    from concourse.bass2jax import _bass_exec_p

    key = (c, tuple(s.key() for s in segs))
    if key in _EXEC_CACHE:
        return _EXEC_CACHE[key]

    bass2jax.install_neuronx_cc_hook()
    nc = _build_nc(segs)
    if not nc.is_finalized():
        nc.finalize()

    in_names, out_names, out_avals, zero_outs = [], [], [], []
    for alloc in nc.m.functions[0].allocations:
        if not isinstance(alloc, mybir.MemoryLocationSet):
            continue
        name = alloc.memorylocations[0].name
        if alloc.kind == "ExternalInput":
            in_names.append(name)
        elif alloc.kind == "ExternalOutput":
            out_names.append(name)
            shape = tuple(alloc.tensor_shape)
            dtype = mybir.dt.np(alloc.dtype)
            out_avals.append(jax.core.ShapedArray(shape, dtype))
            zero_outs.append(np.zeros(shape, dtype))
    n_params = len(in_names)
    all_names = in_names + out_names
    donate = tuple(range(n_params, n_params + len(out_names)))

    def _body(*args):
        outs = _bass_exec_p.bind(
            *args,
            out_avals=tuple(out_avals),
            in_names=tuple(all_names),
            out_names=tuple(out_names),
            lowering_input_output_aliases=(),
            sim_require_finite=True,
            sim_require_nnan=True,
            nc=nc,
        )
        return tuple(outs)

    dev = jax.devices()[c]
    with jax.default_device(dev):
        jf = jax.jit(_body, donate_argnums=donate, keep_unused=True)
        args = _core_args(nc, in_names, zero_outs, in_map, c)
        comp = jf.lower(*args).compile()
    entry = (comp, nc, in_names, out_names, zero_outs)
    _EXEC_CACHE[key] = entry
    return entry


def _core_args(nc, in_names, zero_outs, in_map, c):
    im = dict(in_map)
    if nc.partition_id_tensor is not None:
        im[nc.partition_id_tensor.name] = np.array([[c]], dtype=np.uint32)
    return [np.asarray(im[n]) for n in in_names] + [z.copy() for z in zero_outs]


def _run(attention_weights, input_lengths, output_lengths, ntff_hook=None):
    attention_weights = np.ascontiguousarray(attention_weights, dtype=np.float32)
    il = np.asarray(input_lengths, dtype=np.int64)
    ol = np.asarray(output_lengths, dtype=np.int64)
    assign, core_segs = _build_schedule(il, ol)
    in_maps = []
    for c in range(N_CORES):
        flat = np.empty(FLAT, np.float32)
        flat[:PADF] = 0.0
        flat[PADF : PADF + B_LOC * T * E] = attention_weights[assign[c]].reshape(-1)
        flat[PADF + B_LOC * T * E :] = 0.0
        in_maps.append(
            {"attn": flat, **_make_tables(il, ol, assign[c], core_segs[c])}
        )

    entries = [
        _get_compiled(c, core_segs[c], in_maps[c]) for c in range(N_CORES)
    ]

    def _dispatch():
        futs = []
        for c, (comp, nc, in_names, out_names, zero_outs) in enumerate(entries):
            args = _core_args(nc, in_names, zero_outs, in_maps[c], c)
            futs.append((comp(*args), out_names))
        return [
            {name: np.asarray(v) for name, v in zip(out_names, outs)}
            for outs, out_names in futs
        ]

    if ntff_hook is not None:
        with ntff_hook:
            results = _dispatch()
    else:
        results = _dispatch()

    total = sum(float(r["acc"].sum(dtype=np.float64)) for r in results)
    total -= _garbage_correction(in_maps, il, ol, assign, core_segs)
    return np.float32(total / float(B * T * E)), results


def kernel(attention_weights, input_lengths, output_lengths):
    out, _ = _run(attention_weights, input_lengths, output_lengths)
    return out


# revision 37
# speedup vs baseline: 1.1292x; 1.0572x over previous
"""GuidedAttentionLoss on Trainium2 — 8 NeuronCores, per-core-specialized
diagonal-band gather kernels.

loss = mean(attention_weights * mask), mask[b,i,j] =
    (i < out_len_b) & (j < in_len_b) ? exp(-(j - floor(i/out*in))^2 / (2*0.4^2)) : 0

With sigma=0.4 the Gaussian underflows to exactly 0 in f32 beyond
|j - ideal_i| ~ 4.6, so per valid row only a ~9-wide band of columns
contributes. Each core gets 8 whole batches (greedy-balanced by cost) and its
OWN compiled program specialized to them: per batch a quantized shear line
sigma(i) = a2*(i%128) + at*(i//128) + b tracks ideal(i), and a single 3-dim
DMA access pattern [[400+a2,128],[51200+at,nt],[1,W]] gathers the whole
batch's band ([128 rows/tile] x [W cols], nt tiles) in ONE DMA instruction,
with W fitted exactly (rows past out_len get center=+1e4 and die in the
exp underflow). Per segment:
    path A: d2[:, t] = ACT Square(-w + center_t)   (per-tile bias, no sub)
    path B: d = w - center (DVE, broadcast APs); d2 = ACT Square(d)
    g = ACT Exp(-3.125*d2);  acc[:,s] += g*attn  (DVE stt accum)
Paths are chosen per segment to balance ACT vs DVE. Out-of-range garbage
(front spill j<0, j >= min(in,400)) is not masked on device; the host
subtracts those few boundary terms exactly in f64.

The 8 programs run concurrently: each is jit-compiled for its own device and
dispatched asynchronously; results are gathered and summed on host.
"""

import numpy as np

import concourse.bacc as bacc
import concourse.bass as bass  # noqa: F401
import concourse.mybir as mybir
from concourse.ap import AP
from concourse import tile

N_CORES = 8
B, T, E = 64, 2000, 400
B_LOC = B // N_CORES
P = 128
D = 4
PADF = 512
PADB = 81920
FLAT = PADF + B_LOC * T * E + PADB
NEG_SCALE = -3.125
F32 = mybir.dt.float32
AF = mybir.ActivationFunctionType
OP = mybir.AluOpType

SEG_FIXED_NS = 900.0

_PLAN_CACHE = {}
_EXEC_CACHE = {}


def _ideal_f32(i, in_len, out_len):
    safe_out = np.float32(max(float(out_len), 1.0))
    return np.floor((i.astype(np.float32) / safe_out) * np.float32(in_len)).astype(
        np.float32
    )


def _dma_row_ns(W):
    by = 4 * W
    mult = 2.0 if by < 512 else 1.0
    return max(by * mult / 22.5, 7.0)


class _Seg:
    __slots__ = ("g", "t0", "nt", "W", "a2", "at", "b", "sigma", "path_a")

    def key(self):
        return (self.g, self.t0, self.nt, self.W, self.a2, self.at, self.b,
                self.path_a)


class _BatchCtx:
    """Row band data for a single batch."""

    def __init__(self, b, il, ol):
        self.b = b
        o, n = int(ol[b]), int(il[b])
        self.out = min(o, T)
        self.ntt = (self.out + P - 1) // P
        rows = self.ntt * P
        i = np.arange(rows)
        self.valid = i < self.out
        idl = _ideal_f32(i, n, o).astype(np.float64)
        self.A = np.maximum(0.0, idl - D)
        self.Bb = np.minimum(n - 1, idl + D)
        self.slope = il[b] / max(ol[b], 1)

    def fit(self, t0, t1):
        rows = (t1 - t0) * P
        sl = slice(t0 * P, t1 * P)
        anyv = self.valid[sl]
        if not anyv.any():
            return None
        Amin = self.A[sl]
        Bmax = self.Bb[sl]
        rr = np.arange(rows)
        t_idx = rr // P
        p = rr % P
        s = self.slope
        cands = set()
        for f1 in (np.floor, np.round):
            a2 = int(f1(s))
            for f3 in (np.floor, np.round):
                at3 = int(f3(128 * s))
                for dat in (-1, 0, 1):
                    cands.add((a2, at3 + dat))
        cands.add((0, 0))
        best = None
        for a2, at in cands:
            sig0 = a2 * p + at * t_idx
            b_off = int(np.floor((Amin - sig0)[anyv].min()))
            W = int(np.ceil((Bmax - sig0)[anyv].max() - b_off)) + 1
            if best is None or W < best[0]:
                best = (W, a2, at, b_off)
        return best

    def seg_cost(self, t0, t1):
        f = self.fit(t0, t1)
        if f is None:
            return 0.0, None
        W, a2, at, b_off = f
        nt = t1 - t0
        best = None
        for Wc in {W, 128} if 64 < W < 128 else {W}:
            fw = nt * Wc
            dma = nt * P * _dma_row_ns(Wc) / 16.0
            act = 1.67 * fw + 192 * nt
            dve = 1.042 * fw
            cost = max(dma, act, dve) + 0.25 * (act + dve) + SEG_FIXED_NS
            if best is None or cost < best[0]:
                best = (cost, (Wc, a2, at, b_off))
        return best

    def plan(self):
        nt = self.ntt
        icost = {}
        ifit = {}
        for t0 in range(nt):
            for t1 in range(t0 + 1, nt + 1):
                c, f = self.seg_cost(t0, t1)
                icost[(t0, t1)] = c
                ifit[(t0, t1)] = f
        INF = float("inf")
        dp = [INF] * (nt + 1)
        prev = [0] * (nt + 1)
        dp[0] = 0.0
        for t1 in range(1, nt + 1):
            for t0 in range(t1):
                c = dp[t0] + icost[(t0, t1)]
                if c < dp[t1]:
                    dp[t1] = c
                    prev[t1] = t0
        cuts = []
        t = nt
        while t > 0:
            t0 = prev[t]
            cuts.append((t0, t))
            t = t0
        cuts.reverse()
        out = [(t0, t1, ifit[(t0, t1)]) for t0, t1 in cuts if ifit[(t0, t1)]]
        return dp[nt], out


def _build_schedule(input_lengths, output_lengths):
    """Returns assign[c][g] = batch at core c slot g, and per-core seg lists."""
    il = np.asarray(input_lengths, dtype=np.int64)
    ol = np.asarray(output_lengths, dtype=np.int64)
    ctxs = [_BatchCtx(b, il, ol) for b in range(B)]
    plans = [c.plan() for c in ctxs]
    order = sorted(range(B), key=lambda b: -plans[b][0])
    loads = [0.0] * N_CORES
    slots = [[] for _ in range(N_CORES)]
    for b in order:
        c = min(
            (c for c in range(N_CORES) if len(slots[c]) < B_LOC),
            key=lambda c: loads[c],
        )
        slots[c].append(b)
        loads[c] += plans[b][0]
    assign = [slots[c] for c in range(N_CORES)]

    core_segs = []
    for c in range(N_CORES):
        segs = []
        for g, b in enumerate(assign[c]):
            for t0, t1, (W, a2, at, b_off) in plans[b][1]:
                seg = _Seg()
                seg.g = g
                seg.t0 = t0
                seg.nt = t1 - t0
                seg.W = W
                seg.a2 = a2
                seg.at = at
                seg.b = b_off
                rr = np.arange(seg.nt * P)
                seg.sigma = a2 * (rr % P) + at * (rr // P) + b_off
                segs.append(seg)
        # path A/B to balance ACT vs DVE within this core
        act_ns = dve_ns = 0.0
        for seg in sorted(segs, key=lambda s: -s.nt * s.W):
            fw = seg.nt * seg.W
            a_act = 1.67 * fw + 192 * (seg.nt + 1)
            a_dve = 1.042 * fw + 80
            b_act = 1.67 * fw + 384
            b_dve = 2.084 * fw + 245
            if max(act_ns + a_act, dve_ns + a_dve) <= max(
                act_ns + b_act, dve_ns + b_dve
            ):
                seg.path_a = True
                act_ns += a_act
                dve_ns += a_dve
            else:
                seg.path_a = False
                act_ns += b_act
                dve_ns += b_dve
        _coverage_check(segs, assign[c], il, ol)
        core_segs.append(segs)
    return assign, core_segs


def _coverage_check(segs, assign_c, il, ol):
    covered = {}
    for seg in segs:
        b = assign_c[seg.g]
        o, n = int(ol[b]), int(il[b])
        rows = seg.nt * P
        i = seg.t0 * P + np.arange(rows)
        v = i < min(o, T)
        if v.any():
            idl = _ideal_f32(i, n, o).astype(np.float64)
            A = np.maximum(0.0, idl - D)
            Bb = np.minimum(n - 1, idl + D)
            ok = (~v) | ((seg.sigma <= A) & (Bb < seg.sigma + seg.W))
            assert ok.all(), (seg.g, b, np.where(~ok)[0][:5])
        base = seg.g * T * E + i * E + seg.sigma
        assert (PADF + base).min() >= 0
        assert (PADF + base + seg.W).max() <= FLAT
        cv = covered.setdefault(seg.g, np.zeros(T, bool))
        cv[np.clip(i[v], 0, T - 1)] = True
    for g, b in enumerate(assign_c):
        o = min(int(ol[b]), T)
        cv = covered.get(g, np.zeros(T, bool))
        assert cv[:o].all(), (g, b, int(np.argmin(cv[:o])))


def _build_nc(segs):
    ntt = sum(s.nt for s in segs)
    nseg = len(segs)
    nc = bacc.Bacc(None, target_bir_lowering=False)
    attn = nc.declare_dram_parameter("attn", [FLAT], F32, isOutput=False)
    center_d = nc.declare_dram_parameter("center", [P, ntt], F32, isOutput=False)
    acc_d = nc.declare_dram_parameter("acc", [P, nseg], F32, isOutput=True)

    fwt = sum(s.nt * s.W for s in segs)
    with tile.TileContext(nc) as tc:
        with tc.tile_pool(name="const", bufs=1) as const_pool:
            w_i32 = const_pool.tile([P, E], mybir.dt.int32, tag="w_i32")
            w_f32 = const_pool.tile([P, E], F32, tag="w_f32")
            center = const_pool.tile([P, ntt], F32, tag="center")
            acc = const_pool.tile([P, nseg], F32, tag="acc")
            at_all = const_pool.tile([P, fwt], F32, tag="at_all")
            d_all = const_pool.tile([P, fwt], F32, tag="d_all")
            d2_all = const_pool.tile([P, fwt], F32, tag="d2_all")
            gt_all = const_pool.tile([P, fwt], F32, tag="gt_all")
            jk_all = const_pool.tile([P, fwt], F32, tag="jk_all")

            nc.gpsimd.iota(w_i32[:], pattern=[[1, E]], base=0, channel_multiplier=0)
            nc.vector.tensor_copy(w_f32[:], w_i32[:])
            nc.gpsimd.memset(acc[:], 0.0)

            # center table first: every compute op depends on it
            nc.scalar.dma_start(out=center[:], in_=center_d[:])

            # group segments into chunks; one Square/Exp/reduce per chunk
            chunks = []
            cur = []
            cfw = 0
            for si, seg in enumerate(segs):
                cur.append(si)
                cfw += seg.nt * seg.W
                if cfw >= 640:
                    chunks.append(cur)
                    cur = []
                    cfw = 0
            if cur:
                chunks.append(cur)

            k0s = np.cumsum([0] + [s.nt for s in segs])
            engs = [nc.sync, nc.scalar]
            off = 0
            offs = []
            for si, seg in enumerate(segs):
                offs.append(off)
                nt, W = seg.nt, seg.W
                src = AP(
                    attn[:].tensor,
                    PADF + seg.g * T * E + seg.t0 * P * E + seg.b,
                    [[E + seg.a2, P], [P * E + seg.at, nt], [1, W]],
                )
                dst = at_all[:, off : off + nt * W]
                engs[si % 2].dma_start(
                    out=AP(dst.tensor, dst.offset,
                           [dst.ap[0], [W, nt], [1, W]]),
                    in_=src,
                )
                off += nt * W
            for ci, chunk in enumerate(chunks):
                for si in chunk:
                    seg = segs[si]
                    nt, W = seg.nt, seg.W
                    off = offs[si]
                    # d = w - center (broadcast APs)
                    k0 = int(k0s[si])
                    wap = w_f32[:, 0:W]
                    w_b = AP(wap.tensor, wap.offset,
                             [wap.ap[0], [0, nt], [1, W]])
                    cap = center[:, k0 : k0 + nt]
                    c_b = AP(cap.tensor, cap.offset,
                             [cap.ap[0], [1, nt], [0, W]])
                    dsl = d_all[:, off : off + nt * W]
                    d3 = AP(dsl.tensor, dsl.offset,
                            [dsl.ap[0], [W, nt], [1, W]])
                    nc.vector.tensor_tensor(d3, w_b, c_b, OP.subtract)
                c0 = offs[chunk[0]]
                c1 = offs[chunk[-1]] + segs[chunk[-1]].nt * segs[chunk[-1]].W
                nc.scalar.activation(
                    d2_all[:, c0:c1], d_all[:, c0:c1], AF.Square)
                nc.scalar.activation(
                    gt_all[:, c0:c1], d2_all[:, c0:c1], AF.Exp, scale=NEG_SCALE)
                nc.vector.scalar_tensor_tensor(
                    jk_all[:, c0:c1], gt_all[:, c0:c1], 1.0, at_all[:, c0:c1],
                    OP.mult, OP.mult,
                    accum_out=acc[:, ci : ci + 1],
                )
            nc.sync.dma_start(out=acc_d[:], in_=acc[:])
    return nc


def _make_tables(il, ol, assign_c, segs):
    ntt = sum(s.nt for s in segs)
    center = np.full((P, ntt), 1e4, np.float32)
    k0 = 0
    for seg in segs:
        b = assign_c[seg.g]
        o, n = int(ol[b]), int(il[b])
        rows = seg.nt * P
        i = seg.t0 * P + np.arange(rows)
        idl = _ideal_f32(i, n, o)
        validr = i < min(o, T)
        cen = np.where(validr, idl - seg.sigma.astype(np.float32), np.float32(1e4))
        center[:, k0 : k0 + seg.nt] = cen.reshape(seg.nt, P).T
        k0 += seg.nt
    return {"center": center}


def _garbage_correction(in_maps, il, ol, assign, core_segs):
    M = 24
    corr = 0.0
    for c in range(N_CORES):
        flat = in_maps[c]["attn"]
        for seg in core_segs[c]:
            b = assign[c][seg.g]
            o, n = int(ol[b]), int(il[b])
            lim = min(n, E)
            rows = seg.nt * P
            i = seg.t0 * P + np.arange(rows)
            validr = i < min(o, T)
            idl = _ideal_f32(i, n, o).astype(np.float64)
            sg = seg.sigma
            fr = validr & (
                ((sg < 0) & (idl <= M)) | ((sg + seg.W > lim) & (idl >= lim - M))
            )
            if not fr.any():
                continue
            ii = i[fr]
            j = sg[fr][:, None] + np.arange(seg.W)[None, :]
            dd = j - idl[fr][:, None]
            bad = ((j < 0) | (j >= lim)) & (np.abs(dd) <= M)
            if not bad.any():
                continue
            addr = PADF + seg.g * T * E + ii[:, None] * E + j
            vals = flat[addr[bad]].astype(np.float64)
            corr += float(np.sum(np.exp(-3.125 * dd[bad] ** 2) * vals))
    return corr


def _get_compiled(c, segs, in_map):
    """jit-compile core c's program for device c; cache across calls."""
    import jax
    from concourse import bass2jax
    from concourse.bass2jax import _bass_exec_p

    key = (c, tuple(s.key() for s in segs))
    if key in _EXEC_CACHE:
        return _EXEC_CACHE[key]

    bass2jax.install_neuronx_cc_hook()
    nc = _build_nc(segs)
    if not nc.is_finalized():
        nc.finalize()

    in_names, out_names, out_avals, zero_outs = [], [], [], []
    for alloc in nc.m.functions[0].allocations:
        if not isinstance(alloc, mybir.MemoryLocationSet):
            continue
        name = alloc.memorylocations[0].name
        if alloc.kind == "ExternalInput":
            in_names.append(name)
        elif alloc.kind == "ExternalOutput":
            out_names.append(name)
            shape = tuple(alloc.tensor_shape)
            dtype = mybir.dt.np(alloc.dtype)
            out_avals.append(jax.core.ShapedArray(shape, dtype))
            zero_outs.append(np.zeros(shape, dtype))
    n_params = len(in_names)
    all_names = in_names + out_names
    donate = tuple(range(n_params, n_params + len(out_names)))

    def _body(*args):
        outs = _bass_exec_p.bind(
            *args,
            out_avals=tuple(out_avals),
            in_names=tuple(all_names),
            out_names=tuple(out_names),
            lowering_input_output_aliases=(),
            sim_require_finite=True,
            sim_require_nnan=True,
            nc=nc,
        )
        return tuple(outs)

    dev = jax.devices()[c]
    with jax.default_device(dev):
        jf = jax.jit(_body, donate_argnums=donate, keep_unused=True)
        args = _core_args(nc, in_names, zero_outs, in_map, c)
        comp = jf.lower(*args).compile()
    entry = (comp, nc, in_names, out_names, zero_outs)
    _EXEC_CACHE[key] = entry
    return entry


def _core_args(nc, in_names, zero_outs, in_map, c):
    im = dict(in_map)
    if nc.partition_id_tensor is not None:
        im[nc.partition_id_tensor.name] = np.array([[c]], dtype=np.uint32)
    return [np.asarray(im[n]) for n in in_names] + [z.copy() for z in zero_outs]


def _run(attention_weights, input_lengths, output_lengths, ntff_hook=None):
    attention_weights = np.ascontiguousarray(attention_weights, dtype=np.float32)
    il = np.asarray(input_lengths, dtype=np.int64)
    ol = np.asarray(output_lengths, dtype=np.int64)
    assign, core_segs = _build_schedule(il, ol)
    in_maps = []
    for c in range(N_CORES):
        flat = np.empty(FLAT, np.float32)
        flat[:PADF] = 0.0
        flat[PADF : PADF + B_LOC * T * E] = attention_weights[assign[c]].reshape(-1)
        flat[PADF + B_LOC * T * E :] = 0.0
        in_maps.append(
            {"attn": flat, **_make_tables(il, ol, assign[c], core_segs[c])}
        )

    entries = [
        _get_compiled(c, core_segs[c], in_maps[c]) for c in range(N_CORES)
    ]

    def _dispatch():
        futs = []
        for c, (comp, nc, in_names, out_names, zero_outs) in enumerate(entries):
            args = _core_args(nc, in_names, zero_outs, in_maps[c], c)
            futs.append((comp(*args), out_names))
        return [
            {name: np.asarray(v) for name, v in zip(out_names, outs)}
            for outs, out_names in futs
        ]

    if ntff_hook is not None:
        with ntff_hook:
            results = _dispatch()
    else:
        results = _dispatch()

    total = sum(float(r["acc"].sum(dtype=np.float64)) for r in results)
    total -= _garbage_correction(in_maps, il, ol, assign, core_segs)
    return np.float32(total / float(B * T * E)), results


def kernel(attention_weights, input_lengths, output_lengths):
    out, _ = _run(attention_weights, input_lengths, output_lengths)
    return out


# revision 39
# speedup vs baseline: 1.1410x; 1.0105x over previous
"""GuidedAttentionLoss on Trainium2 — 8 NeuronCores, per-core-specialized
diagonal-band gather kernels.

loss = mean(attention_weights * mask), mask[b,i,j] =
    (i < out_len_b) & (j < in_len_b) ? exp(-(j - floor(i/out*in))^2 / (2*0.4^2)) : 0

With sigma=0.4 the Gaussian underflows to exactly 0 in f32 beyond
|j - ideal_i| ~ 4.6, so per valid row only a ~9-wide band of columns
contributes. Each core gets 8 whole batches (greedy-balanced by cost) and its
OWN compiled program specialized to them: per batch a quantized shear line
sigma(i) = a2*(i%128) + at*(i//128) + b tracks ideal(i), and a single 3-dim
DMA access pattern [[400+a2,128],[51200+at,nt],[1,W]] gathers the whole
batch's band ([128 rows/tile] x [W cols], nt tiles) in ONE DMA instruction,
with W fitted exactly (rows past out_len get center=+1e4 and die in the
exp underflow). Per segment:
    path A: d2[:, t] = ACT Square(-w + center_t)   (per-tile bias, no sub)
    path B: d = w - center (DVE, broadcast APs); d2 = ACT Square(d)
    g = ACT Exp(-3.125*d2);  acc[:,s] += g*attn  (DVE stt accum)
Paths are chosen per segment to balance ACT vs DVE. Out-of-range garbage
(front spill j<0, j >= min(in,400)) is not masked on device; the host
subtracts those few boundary terms exactly in f64.

The 8 programs run concurrently: each is jit-compiled for its own device and
dispatched asynchronously; results are gathered and summed on host.
"""

import numpy as np

import concourse.bacc as bacc
import concourse.bass as bass  # noqa: F401
import concourse.mybir as mybir
from concourse.ap import AP
from concourse import tile

N_CORES = 8
B, T, E = 64, 2000, 400
B_LOC = B // N_CORES
P = 128
D = 4
PADF = 512
PADB = 81920
FLAT = PADF + B_LOC * T * E + PADB
NEG_SCALE = -3.125
F32 = mybir.dt.float32
AF = mybir.ActivationFunctionType
OP = mybir.AluOpType

SEG_FIXED_NS = 900.0

_PLAN_CACHE = {}
_EXEC_CACHE = {}


def _ideal_f32(i, in_len, out_len):
    safe_out = np.float32(max(float(out_len), 1.0))
    return np.floor((i.astype(np.float32) / safe_out) * np.float32(in_len)).astype(
        np.float32
    )


def _dma_row_ns(W):
    by = 4 * W
    mult = 2.0 if by < 512 else 1.0
    return max(by * mult / 22.5, 7.0)


class _Seg:
    __slots__ = ("g", "t0", "nt", "W", "a2", "at", "b", "sigma", "path_a")

    def key(self):
        return (self.g, self.t0, self.nt, self.W, self.a2, self.at, self.b,
                self.path_a)


class _BatchCtx:
    """Row band data for a single batch."""

    def __init__(self, b, il, ol):
        self.b = b
        o, n = int(ol[b]), int(il[b])
        self.out = min(o, T)
        self.ntt = (self.out + P - 1) // P
        rows = self.ntt * P
        i = np.arange(rows)
        self.valid = i < self.out
        idl = _ideal_f32(i, n, o).astype(np.float64)
        self.A = np.maximum(0.0, idl - D)
        self.Bb = np.minimum(n - 1, idl + D)
        self.slope = il[b] / max(ol[b], 1)

    def fit(self, t0, t1):
        rows = (t1 - t0) * P
        sl = slice(t0 * P, t1 * P)
        anyv = self.valid[sl]
        if not anyv.any():
            return None
        Amin = self.A[sl]
        Bmax = self.Bb[sl]
        rr = np.arange(rows)
        t_idx = rr // P
        p = rr % P
        s = self.slope
        cands = set()
        for f1 in (np.floor, np.round):
            a2 = int(f1(s))
            for f3 in (np.floor, np.round):
                at3 = int(f3(128 * s))
                for dat in (-1, 0, 1):
                    cands.add((a2, at3 + dat))
        cands.add((0, 0))
        best = None
        for a2, at in cands:
            sig0 = a2 * p + at * t_idx
            b_off = int(np.floor((Amin - sig0)[anyv].min()))
            W = int(np.ceil((Bmax - sig0)[anyv].max() - b_off)) + 1
            if best is None or W < best[0]:
                best = (W, a2, at, b_off)
        return best

    def seg_cost(self, t0, t1):
        f = self.fit(t0, t1)
        if f is None:
            return 0.0, None
        W, a2, at, b_off = f
        nt = t1 - t0
        best = None
        for Wc in {W, 128} if 64 < W < 128 else {W}:
            fw = nt * Wc
            dma = nt * P * _dma_row_ns(Wc) / 16.0
            act = 1.67 * fw + 192 * nt
            dve = 1.042 * fw
            cost = max(dma, act, dve) + 0.25 * (act + dve) + SEG_FIXED_NS
            if best is None or cost < best[0]:
                best = (cost, (Wc, a2, at, b_off))
        return best

    def plan(self):
        nt = self.ntt
        icost = {}
        ifit = {}
        for t0 in range(nt):
            for t1 in range(t0 + 1, nt + 1):
                c, f = self.seg_cost(t0, t1)
                icost[(t0, t1)] = c
                ifit[(t0, t1)] = f
        INF = float("inf")
        dp = [INF] * (nt + 1)
        prev = [0] * (nt + 1)
        dp[0] = 0.0
        for t1 in range(1, nt + 1):
            for t0 in range(t1):
                c = dp[t0] + icost[(t0, t1)]
                if c < dp[t1]:
                    dp[t1] = c
                    prev[t1] = t0
        cuts = []
        t = nt
        while t > 0:
            t0 = prev[t]
            cuts.append((t0, t))
            t = t0
        cuts.reverse()
        out = [(t0, t1, ifit[(t0, t1)]) for t0, t1 in cuts if ifit[(t0, t1)]]
        return dp[nt], out


def _build_schedule(input_lengths, output_lengths):
    """Returns assign[c][g] = batch at core c slot g, and per-core seg lists."""
    il = np.asarray(input_lengths, dtype=np.int64)
    ol = np.asarray(output_lengths, dtype=np.int64)
    ctxs = [_BatchCtx(b, il, ol) for b in range(B)]
    plans = [c.plan() for c in ctxs]
    order = sorted(range(B), key=lambda b: -plans[b][0])
    loads = [0.0] * N_CORES
    slots = [[] for _ in range(N_CORES)]
    for b in order:
        c = min(
            (c for c in range(N_CORES) if len(slots[c]) < B_LOC),
            key=lambda c: loads[c],
        )
        slots[c].append(b)
        loads[c] += plans[b][0]
    assign = [slots[c] for c in range(N_CORES)]

    core_segs = []
    for c in range(N_CORES):
        segs = []
        for g, b in enumerate(assign[c]):
            for t0, t1, (W, a2, at, b_off) in plans[b][1]:
                seg = _Seg()
                seg.g = g
                seg.t0 = t0
                seg.nt = t1 - t0
                seg.W = W
                seg.a2 = a2
                seg.at = at
                seg.b = b_off
                rr = np.arange(seg.nt * P)
                seg.sigma = a2 * (rr % P) + at * (rr // P) + b_off
                segs.append(seg)
        # path A/B to balance ACT vs DVE within this core
        act_ns = dve_ns = 0.0
        for seg in sorted(segs, key=lambda s: -s.nt * s.W):
            fw = seg.nt * seg.W
            a_act = 1.67 * fw + 192 * (seg.nt + 1)
            a_dve = 1.042 * fw + 80
            b_act = 1.67 * fw + 384
            b_dve = 2.084 * fw + 245
            if max(act_ns + a_act, dve_ns + a_dve) <= max(
                act_ns + b_act, dve_ns + b_dve
            ):
                seg.path_a = True
                act_ns += a_act
                dve_ns += a_dve
            else:
                seg.path_a = False
                act_ns += b_act
                dve_ns += b_dve
        _coverage_check(segs, assign[c], il, ol)
        core_segs.append(segs)
    return assign, core_segs


def _coverage_check(segs, assign_c, il, ol):
    covered = {}
    for seg in segs:
        b = assign_c[seg.g]
        o, n = int(ol[b]), int(il[b])
        rows = seg.nt * P
        i = seg.t0 * P + np.arange(rows)
        v = i < min(o, T)
        if v.any():
            idl = _ideal_f32(i, n, o).astype(np.float64)
            A = np.maximum(0.0, idl - D)
            Bb = np.minimum(n - 1, idl + D)
            ok = (~v) | ((seg.sigma <= A) & (Bb < seg.sigma + seg.W))
            assert ok.all(), (seg.g, b, np.where(~ok)[0][:5])
        base = seg.g * T * E + i * E + seg.sigma
        assert (PADF + base).min() >= 0
        assert (PADF + base + seg.W).max() <= FLAT
        cv = covered.setdefault(seg.g, np.zeros(T, bool))
        cv[np.clip(i[v], 0, T - 1)] = True
    for g, b in enumerate(assign_c):
        o = min(int(ol[b]), T)
        cv = covered.get(g, np.zeros(T, bool))
        assert cv[:o].all(), (g, b, int(np.argmin(cv[:o])))


def _build_nc(segs):
    ntt = sum(s.nt for s in segs)
    nseg = len(segs)
    nc = bacc.Bacc(None, target_bir_lowering=False)
    attn = nc.declare_dram_parameter("attn", [FLAT], F32, isOutput=False)
    center_d = nc.declare_dram_parameter("center", [P, ntt + E], F32, isOutput=False)
    acc_d = nc.declare_dram_parameter("acc", [P, nseg], F32, isOutput=True)

    fwt = sum(s.nt * s.W for s in segs)
    with tile.TileContext(nc) as tc:
        with tc.tile_pool(name="const", bufs=1) as const_pool:
            centw = const_pool.tile([P, ntt + E], F32, tag="centw")
            center = centw[:, 0:ntt]
            w_f32 = centw[:, ntt : ntt + E]
            acc = const_pool.tile([P, nseg], F32, tag="acc")
            at_all = const_pool.tile([P, fwt], F32, tag="at_all")
            d_all = const_pool.tile([P, fwt], F32, tag="d_all")
            d2_all = const_pool.tile([P, fwt], F32, tag="d2_all")
            gt_all = const_pool.tile([P, fwt], F32, tag="gt_all")
            jk_all = const_pool.tile([P, fwt], F32, tag="jk_all")

            nc.gpsimd.memset(acc[:], 0.0)

            # center+w table first: every compute op depends on it
            nc.scalar.dma_start(out=centw[:], in_=center_d[:])

            # group segments into chunks; one Square/Exp/reduce per chunk
            chunks = []
            cur = []
            cfw = 0
            for si, seg in enumerate(segs):
                cur.append(si)
                cfw += seg.nt * seg.W
                if cfw >= 448:
                    chunks.append(cur)
                    cur = []
                    cfw = 0
            if cur:
                chunks.append(cur)

            k0s = np.cumsum([0] + [s.nt for s in segs])
            engs = [nc.sync, nc.scalar]
            off = 0
            offs = []
            for si, seg in enumerate(segs):
                offs.append(off)
                nt, W = seg.nt, seg.W
                src = AP(
                    attn[:].tensor,
                    PADF + seg.g * T * E + seg.t0 * P * E + seg.b,
                    [[E + seg.a2, P], [P * E + seg.at, nt], [1, W]],
                )
                dst = at_all[:, off : off + nt * W]
                (nc.scalar if si % 3 == 2 else nc.sync).dma_start(
                    out=AP(dst.tensor, dst.offset,
                           [dst.ap[0], [W, nt], [1, W]]),
                    in_=src,
                )
                off += nt * W
            for ci, chunk in enumerate(chunks):
                for si in chunk:
                    seg = segs[si]
                    nt, W = seg.nt, seg.W
                    off = offs[si]
                    # d = w - center (broadcast APs)
                    k0 = int(k0s[si])
                    wap = w_f32[:, 0:W]
                    w_b = AP(wap.tensor, wap.offset,
                             [wap.ap[0], [0, nt], [1, W]])
                    cap = center[:, k0 : k0 + nt]
                    c_b = AP(cap.tensor, cap.offset,
                             [cap.ap[0], [1, nt], [0, W]])
                    dsl = d_all[:, off : off + nt * W]
                    d3 = AP(dsl.tensor, dsl.offset,
                            [dsl.ap[0], [W, nt], [1, W]])
                    nc.vector.tensor_tensor(d3, w_b, c_b, OP.subtract)
                c0 = offs[chunk[0]]
                c1 = offs[chunk[-1]] + segs[chunk[-1]].nt * segs[chunk[-1]].W
                nc.scalar.activation(
                    d2_all[:, c0:c1], d_all[:, c0:c1], AF.Square)
                nc.scalar.activation(
                    gt_all[:, c0:c1], d2_all[:, c0:c1], AF.Exp, scale=NEG_SCALE)
                nc.vector.scalar_tensor_tensor(
                    jk_all[:, c0:c1], gt_all[:, c0:c1], 1.0, at_all[:, c0:c1],
                    OP.mult, OP.mult,
                    accum_out=acc[:, ci : ci + 1],
                )
            nc.sync.dma_start(out=acc_d[:], in_=acc[:])
    return nc


def _make_tables(il, ol, assign_c, segs):
    ntt = sum(s.nt for s in segs)
    centw = np.full((P, ntt + E), 1e4, np.float32)
    k0 = 0
    for seg in segs:
        b = assign_c[seg.g]
        o, n = int(ol[b]), int(il[b])
        rows = seg.nt * P
        i = seg.t0 * P + np.arange(rows)
        idl = _ideal_f32(i, n, o)
        validr = i < min(o, T)
        cen = np.where(validr, idl - seg.sigma.astype(np.float32), np.float32(1e4))
        centw[:, k0 : k0 + seg.nt] = cen.reshape(seg.nt, P).T
        k0 += seg.nt
    centw[:, ntt : ntt + E] = np.arange(E, dtype=np.float32)[None, :]
    return {"center": centw}


def _garbage_correction(in_maps, il, ol, assign, core_segs):
    M = 24
    corr = 0.0
    for c in range(N_CORES):
        flat = in_maps[c]["attn"]
        for seg in core_segs[c]:
            b = assign[c][seg.g]
            o, n = int(ol[b]), int(il[b])
            lim = min(n, E)
            rows = seg.nt * P
            i = seg.t0 * P + np.arange(rows)
            validr = i < min(o, T)
            idl = _ideal_f32(i, n, o).astype(np.float64)
            sg = seg.sigma
            fr = validr & (
                ((sg < 0) & (idl <= M)) | ((sg + seg.W > lim) & (idl >= lim - M))
            )
            if not fr.any():
                continue
            ii = i[fr]
            j = sg[fr][:, None] + np.arange(seg.W)[None, :]
            dd = j - idl[fr][:, None]
            bad = ((j < 0) | (j >= lim)) & (np.abs(dd) <= M)
            if not bad.any():
                continue
            addr = PADF + seg.g * T * E + ii[:, None] * E + j
            vals = flat[addr[bad]].astype(np.float64)
            corr += float(np.sum(np.exp(-3.125 * dd[bad] ** 2) * vals))
    return corr


def _get_compiled(c, segs, in_map):
    """jit-compile core c's program for device c; cache across calls."""
    import jax
    from concourse import bass2jax
    from concourse.bass2jax import _bass_exec_p

    key = (c, tuple(s.key() for s in segs))
    if key in _EXEC_CACHE:
        return _EXEC_CACHE[key]

    bass2jax.install_neuronx_cc_hook()
    nc = _build_nc(segs)
    if not nc.is_finalized():
        nc.finalize()

    in_names, out_names, out_avals, zero_outs = [], [], [], []
    for alloc in nc.m.functions[0].allocations:
        if not isinstance(alloc, mybir.MemoryLocationSet):
            continue
        name = alloc.memorylocations[0].name
        if alloc.kind == "ExternalInput":
            in_names.append(name)
        elif alloc.kind == "ExternalOutput":
            out_names.append(name)
            shape = tuple(alloc.tensor_shape)
            dtype = mybir.dt.np(alloc.dtype)
            out_avals.append(jax.core.ShapedArray(shape, dtype))
            zero_outs.append(np.zeros(shape, dtype))
    n_params = len(in_names)
    all_names = in_names + out_names
    donate = tuple(range(n_params, n_params + len(out_names)))

    def _body(*args):
        outs = _bass_exec_p.bind(
            *args,
            out_avals=tuple(out_avals),
            in_names=tuple(all_names),
            out_names=tuple(out_names),
            lowering_input_output_aliases=(),
            sim_require_finite=True,
            sim_require_nnan=True,
            nc=nc,
        )
        return tuple(outs)

    dev = jax.devices()[c]
    with jax.default_device(dev):
        jf = jax.jit(_body, donate_argnums=donate, keep_unused=True)
        args = _core_args(nc, in_names, zero_outs, in_map, c)
        comp = jf.lower(*args).compile()
    entry = (comp, nc, in_names, out_names, zero_outs)
    _EXEC_CACHE[key] = entry
    return entry


def _core_args(nc, in_names, zero_outs, in_map, c):
    im = dict(in_map)
    if nc.partition_id_tensor is not None:
        im[nc.partition_id_tensor.name] = np.array([[c]], dtype=np.uint32)
    return [np.asarray(im[n]) for n in in_names] + [z.copy() for z in zero_outs]


def _run(attention_weights, input_lengths, output_lengths, ntff_hook=None):
    attention_weights = np.ascontiguousarray(attention_weights, dtype=np.float32)
    il = np.asarray(input_lengths, dtype=np.int64)
    ol = np.asarray(output_lengths, dtype=np.int64)
    assign, core_segs = _build_schedule(il, ol)
    in_maps = []
    for c in range(N_CORES):
        flat = np.empty(FLAT, np.float32)
        flat[:PADF] = 0.0
        flat[PADF : PADF + B_LOC * T * E] = attention_weights[assign[c]].reshape(-1)
        flat[PADF + B_LOC * T * E :] = 0.0
        in_maps.append(
            {"attn": flat, **_make_tables(il, ol, assign[c], core_segs[c])}
        )

    entries = [
        _get_compiled(c, core_segs[c], in_maps[c]) for c in range(N_CORES)
    ]

    def _dispatch():
        futs = []
        for c, (comp, nc, in_names, out_names, zero_outs) in enumerate(entries):
            args = _core_args(nc, in_names, zero_outs, in_maps[c], c)
            futs.append((comp(*args), out_names))
        return [
            {name: np.asarray(v) for name, v in zip(out_names, outs)}
            for outs, out_names in futs
        ]

    if ntff_hook is not None:
        with ntff_hook:
            results = _dispatch()
    else:
        results = _dispatch()

    total = sum(float(r["acc"].sum(dtype=np.float64)) for r in results)
    total -= _garbage_correction(in_maps, il, ol, assign, core_segs)
    return np.float32(total / float(B * T * E)), results


def kernel(attention_weights, input_lengths, output_lengths):
    out, _ = _run(attention_weights, input_lengths, output_lengths)
    return out
